# revision 1
# baseline (speedup 1.0000x reference)
"""Trainium2 Bass kernel for nn_Conv_39333310497378 (nms_detection).

Reference computation:
  x [16384, 1, 41, 40] f32, W [9, 50, 1, 6, 40] f32
  9 overlapping height-sections of x (section i = rows 4i..4i+8), each conv'd
  with its own [50, 1, 6, 40] kernel (VALID) -> [B, 50, 4, 1], max-pooled over
  the 4 -> [B, 50, 1, 1]; concat sections -> pots [B, 50, 9, 1];
  spks = (pots > 6.2) as 1.0/0.0.

Strategy (pure data parallelism over batch, 8 cores x 2048 samples):
  All 36 conv outputs j (= 4*sec + h) are dot-products of 240 consecutive
  elements of the flattened per-sample x row-block (elements 40j .. 40j+239)
  with per-(j, out-channel) weights.  Per core, x is staged host-side as a
  transposed [1664, 2048] bf16 array (flattened element-index major, batch
  minor, padded 1640 -> 13*128).  Each 128-element chunk c becomes a matmul
  stationary operand [128, 128-batch-tile]; a host-precomputed banded weight
  tile Wb[c] [128, <=450] (moving operand) scatters the chunk's contribution
  into PSUM columns 50j+o for every j whose input window overlaps the chunk.
  PSUM accumulates across the 13 chunks (per-element has_written semantics;
  start=True on the first matmul touching each 512-col bank).  VectorE then
  max-reduces h (groups of 4 columns-of-50), writes pots in [o, sec] layout,
  thresholds for spks, and both DMA out contiguously.
"""
import math
import sys

import numpy as np

sys.path.insert(0, "/opt/trn_rl_repo")

import ml_dtypes  # noqa: E402

import concourse.bass as bass  # noqa: E402
import concourse.mybir as mybir  # noqa: E402
import concourse.tile as tile  # noqa: E402
from concourse import bacc  # noqa: E402
from concourse.bass_utils import run_bass_kernel_spmd  # noqa: E402

BF16 = mybir.dt.bfloat16
F32 = mybir.dt.float32

B, ROWS, WIDTH = 16384, 41, 40
NSEC, OC = 9, 50
NJ = 36
THRESHOLD = 6.2
NCORES = 8
BC = B // NCORES            # 2048 samples per core
E = ROWS * WIDTH            # 1640 elements per sample
NCHUNK = 13
EP = NCHUNK * 128           # 1664 (padded)
BT = 128                    # batch tile = psum partition dim
GRP = 4                     # batch tiles per DMA group
PSUM_COLS = 2048            # 4 banks


def _windows():
    jlo, jhi = [], []
    for c in range(NCHUNK):
        js = [j for j in range(NJ)
              if 40 * j < 128 * c + 128 and 40 * j + 240 > 128 * c]
        jlo.append(min(js)); jhi.append(max(js))
    return jlo, jhi


def _segments(jlo, jhi):
    """Matmul segments in emission order: (chunk, col_a, col_b, start, stop).

    PSUM accumulate flags are only per-element in principle; both CoreSim and
    the safe HW model require each matmul to be wholly first-write (pending
    zero) or wholly accumulate within its bank.  Chunk col-windows have
    nondecreasing ends, so each chunk/bank intersection splits into a "fresh"
    piece (cols beyond everything written so far in the bank) and an
    "accumulate" piece.  Only the very first matmul of a bank carries
    start=True (it marks the entire bank pending-zero).
    """
    nbanks = math.ceil(NJ * OC / 512)
    prev_hi = [512 * k for k in range(nbanks)]
    bank_started = [False] * nbanks
    pieces = []
    for c in range(NCHUNK):
        A, Bc = jlo[c] * OC, (jhi[c] + 1) * OC
        for k in range(nbanks):
            lo, hi = max(A, 512 * k), min(Bc, 512 * (k + 1))
            if lo >= hi:
                continue
            old_hi = prev_hi[k]
            assert lo <= old_hi, f"coverage gap in bank {k}: {lo} > {old_hi}"
            if hi > old_hi:                       # fresh columns
                pieces.append([c, old_hi, hi, not bank_started[k], False])
                bank_started[k] = True
                prev_hi[k] = hi
            if lo < min(hi, old_hi):              # accumulate columns
                pieces.append([c, lo, min(hi, old_hi), False, False])
    last = {}
    for idx, p in enumerate(pieces):
        last[p[1] // 512] = idx
    for idx in last.values():
        pieces[idx][4] = True
    return [tuple(p) for p in pieces]


def _build_wband(W, jlo, jhi):
    """[NCHUNK, 128, 450] f32 banded weights; col (j-jlo)*50+o, row = elem-128c."""
    Wsq = np.asarray(W, np.float32)[:, :, 0]          # [9, 50, 6, 40]
    Wb = np.zeros((NCHUNK, 128, 450), np.float32)
    for c in range(NCHUNK):
        for j in range(jlo[c], jhi[c] + 1):
            sec = j // 4
            e0, e1 = max(40 * j, 128 * c), min(40 * j + 240, 128 * (c + 1))
            es = np.arange(e0, e1)
            Wb[c, es - 128 * c, (j - jlo[c]) * OC:(j - jlo[c] + 1) * OC] = \
                Wsq[sec][:, es // 40 - j, es % 40].T
    return Wb


def _build_program(bc=BC):
    """One-core SPMD program operating on a [EP, bc] transposed x shard."""
    jlo, jhi = _windows()
    segs = _segments(jlo, jhi)
    n_bt = bc // BT
    n_grp = max(1, n_bt // GRP)
    grp = n_bt // n_grp

    ob = 2 if n_bt % 2 == 0 else 1          # batch tiles per output DMA

    nc = bacc.Bacc(None)
    xT_d = nc.dram_tensor("xT", [NCHUNK, 128, bc], BF16, kind="ExternalInput")
    wb_d = nc.dram_tensor("Wb", [128, NCHUNK, 450], BF16, kind="ExternalInput")
    pots_d = nc.dram_tensor("pots", [n_bt, BT, OC * NSEC], BF16,
                            kind="ExternalOutput")
    spks_d = nc.dram_tensor("spks", [n_bt, BT, OC * NSEC], BF16,
                            kind="ExternalOutput")

    with tile.TileContext(nc) as tc:
        with (
            tc.tile_pool(name="w", bufs=1) as wpool,
            tc.tile_pool(name="x", bufs=3) as xpool,
            tc.tile_pool(name="out", bufs=2) as opool,
            tc.tile_pool(name="ps", bufs=2, space="PSUM") as pspool,
        ):
            # weights go on the ACT HWDGE ring so the sync ring starts on x
            wtile = wpool.tile([128, NCHUNK, 450], BF16)
            nc.scalar.dma_start(wtile[:], wb_d[:])
            nthr = wpool.tile([128, 1], F32, tag="nthr")
            nc.any.memset(nthr[:], -THRESHOLD)
            po = sp2 = None
            x0 = None
            for g in range(n_grp):
                if g == 0:
                    # group 0 loads per-chunk so PE can start on chunk 0
                    # while the rest is still in flight
                    x0 = [xpool.tile([128, grp * BT], BF16, tag=f"x0_{c}",
                                     name=f"x0_{c}")
                          for c in range(NCHUNK)]
                    for c in range(NCHUNK):
                        nc.sync.dma_start(x0[c][:], xT_d[c, :, 0:grp * BT])
                else:
                    # one fused DMA for all 13 chunks of this batch group
                    xg = xpool.tile([128, NCHUNK, grp * BT], BF16, tag="xg")
                    nc.sync.dma_start(
                        xg[:],
                        xT_d[:, :, g * grp * BT:(g + 1) * grp * BT]
                        .rearrange("c p b -> p c b"))
                for tl in range(grp):
                    bt = g * grp + tl
                    s = bt % ob
                    if s == 0:
                        po = opool.tile([128, ob, OC * NSEC], BF16, tag="po")
                        sp2 = opool.tile([128, ob, OC * NSEC], BF16, tag="sp")
                    ps = pspool.tile([128, PSUM_COLS], F32, tag="ps")
                    for (c, a, b, st, stp) in segs:
                        lhsT = (x0[c][:, tl * BT:(tl + 1) * BT] if g == 0
                                else xg[:, c, tl * BT:(tl + 1) * BT])
                        nc.tensor.matmul(
                            ps[:, a:b], lhsT,
                            wtile[:, c, a - jlo[c] * OC: b - jlo[c] * OC],
                            start=st, stop=stp)
                    # [p, i, o, h] view, h innermost -> one reduce_max over X
                    psv = ps[:, :NJ * OC].rearrange(
                        "p (i h o) -> p i o h", h=4, o=OC)
                    pov = po[:, s, :].rearrange("p (o i) -> p i o", i=NSEC)
                    nc.vector.tensor_reduce(
                        pov, psv, axis=mybir.AxisListType.X,
                        op=mybir.AluOpType.max)
                    # spks = Relu(Sign(pots - thr)) on the otherwise-idle ACT
                    nc.scalar.activation(
                        sp2[:, s, :], po[:, s, :],
                        mybir.ActivationFunctionType.Sign, bias=nthr[:])
                    nc.scalar.activation(
                        sp2[:, s, :], sp2[:, s, :],
                        mybir.ActivationFunctionType.Relu)
                    if s == ob - 1:
                        # stores go out on the ACT HWDGE ring so input
                        # prefetch never queues behind them on the sync ring
                        t0 = bt - (ob - 1)
                        nc.scalar.dma_start(
                            pots_d[t0:t0 + ob].rearrange("t p n -> p t n"),
                            po[:])
                        nc.scalar.dma_start(
                            spks_d[t0:t0 + ob].rearrange("t p n -> p t n"),
                            sp2[:])
    nc.compile()
    return nc


_PROGRAM_CACHE = {}


def _get_program(bc=BC):
    if bc not in _PROGRAM_CACHE:
        _PROGRAM_CACHE[bc] = _build_program(bc)
    return _PROGRAM_CACHE[bc]


def _prep_inputs(x, W):
    jlo, jhi = _windows()
    wb = _build_wband(W, jlo, jhi).transpose(1, 0, 2)       # [128, 13, 450]
    wb = np.ascontiguousarray(wb).astype(ml_dtypes.bfloat16)
    xf = np.asarray(x, np.float32).reshape(B, E)
    in_maps = []
    for ci in range(NCORES):
        xs = xf[ci * BC:(ci + 1) * BC]
        xpad = np.zeros((BC, EP), np.float32)
        xpad[:, :E] = xs
        xT = np.ascontiguousarray(xpad.T).astype(ml_dtypes.bfloat16)
        in_maps.append({"xT": xT.reshape(NCHUNK, 128, BC), "Wb": wb})
    return in_maps


def kernel(x, W):
    nc = _get_program()
    in_maps = _prep_inputs(x, W)
    res = run_bass_kernel_spmd(nc, in_maps, list(range(NCORES)))
    pots = np.concatenate(
        [np.asarray(r["pots"]).astype(np.float32).reshape(BC, OC * NSEC)
         for r in res.results], axis=0)
    spks = np.concatenate(
        [np.asarray(r["spks"]).astype(np.float32).reshape(BC, OC * NSEC)
         for r in res.results], axis=0)
    pots = pots.reshape(B, OC, NSEC, 1)
    spks = spks.reshape(B, OC, NSEC, 1)
    return pots, spks



# revision 3
# speedup vs baseline: 1.1572x; 1.1572x over previous
"""Trainium2 Bass kernel for nn_Conv_39333310497378 (nms_detection), v2.

Reference computation:
  x [16384, 1, 41, 40] f32, W [9, 50, 1, 6, 40] f32
  9 overlapping height-sections of x (section i = rows 4i..4i+8), each conv'd
  with its own [50, 1, 6, 40] kernel (VALID) -> [B, 50, 4, 1], max-pooled over
  the 4 -> [B, 50, 1, 1]; concat sections -> pots [B, 50, 9, 1];
  spks = (pots > 6.2) as 1.0/0.0.

v2 strategy (vs v1 bf16 baseline at ~69.5us):
  * fp8(e4m3) inputs/weights (numerically validated: rel err ~1.0e-2 vs the
    2e-2 gate).  x staged as 13 transposed k-tiles of 128 elements
    [128, 13, bc]; banded weights scatter each k-tile's contribution into
    psum columns j*50+o.  MODE selects the matmul flavor:
      - "fp8":  13 single-k-tile matmuls/tile, FWL weight loads (fast,
                contiguous), 5200 streamed cols/tile at 1 col/cycle.
      - "dr":   6 DoubleRow pairs + 1 single, 3400 cols/tile at 2x MACs
                but slow non-FWL LDWEIGHTS.
      - "drsw": DoubleRowSwInterleave — host pre-interleaves the stationary
                x pairs so weight loads read contiguously.
  * Output: bf16 pots + fp8 spks; pots columns are i-major (sec*50+o), host
    transposes to the reference [B, 50, 9, 1] layout.
  * DMA: fp8 x in 28 (pair, batch-quarter) pieces so the PE can start after
    ~1/7 of the input is resident.  ~6.5MB/core total vs ~12MB in v1.
"""
import math
import os
import sys

import numpy as np

sys.path.insert(0, "/opt/trn_rl_repo")

import ml_dtypes  # noqa: E402

import concourse.bass as bass  # noqa: E402
import concourse.mybir as mybir  # noqa: E402
import concourse.tile as tile  # noqa: E402
from concourse import bacc  # noqa: E402
from concourse.bass_utils import run_bass_kernel_spmd  # noqa: E402

FP8 = mybir.dt.float8e4
BF16 = mybir.dt.bfloat16
F32 = mybir.dt.float32
NP_FP8 = ml_dtypes.float8_e4m3

B, ROWS, WIDTH = 16384, 41, 40
NSEC, OC = 9, 50
NJ = 36
THRESHOLD = 6.2
NCORES = 8
BC = B // NCORES            # 2048 samples per core
E = ROWS * WIDTH            # 1640 elements per sample
NKT = 13                    # 128-element k-tiles
EP = NKT * 128              # 1664 (padded)
BT = 128                    # batch tile = psum partition dim
PSUM_COLS = 2048            # 4 banks

MODE = os.environ.get("KMODE", "dr")        # "fp8" | "dr" | "drsw"


def _units(mode=None):
    """Matmul units: (jlo, jhi, ktiles) covering the element range of the
    unit's k-tiles.  "fp8": 13 singles; "dr"/"drsw": 6 pairs + 1 single."""
    mode = mode or MODE
    units = []
    if mode == "fp8":
        groups = [(kt,) for kt in range(NKT)]
    else:
        groups = [(2 * c, 2 * c + 1) for c in range(6)] + [(12,)]
    for kts in groups:
        e0, e1 = 128 * kts[0], 128 * (kts[-1] + 1)
        js = [j for j in range(NJ) if 40 * j < e1 and 40 * j + 240 > e0]
        units.append((min(js), max(js), kts))
    return units


def _segments(units):
    """Emission-order matmul pieces: (unit, col_a, col_b, start, stop).

    PSUM accumulate flags must be wholly-fresh or wholly-accumulate per
    piece within a 512-col bank; unit col windows have nondecreasing ends
    so each unit/bank intersection splits into a fresh piece and an
    accumulate piece.  Only the first matmul of a bank carries start=True
    (it marks the entire bank pending-zero).
    """
    nbanks = math.ceil(NJ * OC / 512)
    prev_hi = [512 * k for k in range(nbanks)]
    bank_started = [False] * nbanks
    pieces = []
    for u, (jlo, jhi, _) in enumerate(units):
        A, Bc = jlo * OC, (jhi + 1) * OC
        for k in range(nbanks):
            lo, hi = max(A, 512 * k), min(Bc, 512 * (k + 1))
            if lo >= hi:
                continue
            old_hi = prev_hi[k]
            assert lo <= old_hi, f"coverage gap in bank {k}: {lo} > {old_hi}"
            if hi > old_hi:                       # fresh columns
                pieces.append([u, old_hi, hi, not bank_started[k], False])
                bank_started[k] = True
                prev_hi[k] = hi
            if lo < min(hi, old_hi):              # accumulate columns
                pieces.append([u, lo, min(hi, old_hi), False, False])
    last = {}
    for idx, p in enumerate(pieces):
        last[p[1] // 512] = idx
    for idx in last.values():
        pieces[idx][4] = True
    return [tuple(p) for p in pieces]


def _build_wband(W, units):
    """Packed banded fp8 weights [128, total] (pair tiles are t-major
    [128, nkt*ncols]; singles are [128, ncols])."""
    Wsq = np.asarray(W, np.float32)[:, :, 0]          # [9, 50, 6, 40]
    offs, total = [], 0
    for (jlo, jhi, kts) in units:
        offs.append(total)
        total += len(kts) * (jhi - jlo + 1) * OC
    offs.append(total)
    Wb = np.zeros((128, total), np.float32)
    for u, (jlo, jhi, kts) in enumerate(units):
        ncols = (jhi - jlo + 1) * OC
        for t, kt in enumerate(kts):
            for j in range(jlo, jhi + 1):
                sec = j // 4
                e0 = max(40 * j, 128 * kt)
                e1 = min(40 * j + 240, 128 * kt + 128, E)
                if e0 >= e1:
                    continue
                es = np.arange(e0, e1)
                cols = offs[u] + t * ncols + (j - jlo) * OC + np.arange(OC)
                Wb[np.ix_(es - 128 * kt, cols)] = \
                    Wsq[sec][:, es // 40 - j, es % 40].T
    return Wb.astype(NP_FP8), offs, total


def _build_program(bc=BC, mode=None):
    """One-core SPMD program operating on a transposed fp8 x shard."""
    mode = mode or MODE
    units = _units(mode)
    segs = _segments(units)
    _, offs, wtotal = _build_wband(np.zeros((NSEC, OC, 1, 6, WIDTH)), units)
    n_bt = bc // BT
    nq = min(4, n_bt)           # batch quarters for input DMA granularity
    tpq = n_bt // nq            # tiles per quarter
    qs = bc // nq               # samples per quarter

    ob = 2 if n_bt % 2 == 0 else 1          # batch tiles per output DMA

    # x dram layout: "drsw" pre-interleaves stationary pairs per batch tile
    # -> [128, nunits, tiles, 256]; otherwise k-tile planes [128, NKT, bc].
    nU = len(units)

    nc = bacc.Bacc(None)
    if mode == "drsw":
        xT_d = nc.dram_tensor("xT", [128, nU, n_bt, 256], FP8,
                              kind="ExternalInput")
    else:
        xT_d = nc.dram_tensor("xT", [128, NKT, bc], FP8, kind="ExternalInput")
    wb_d = nc.dram_tensor("Wb", [128, wtotal], FP8, kind="ExternalInput")
    pots_d = nc.dram_tensor("pots", [n_bt, BT, OC * NSEC], BF16,
                            kind="ExternalOutput")
    spks_d = nc.dram_tensor("spks", [n_bt, BT, OC * NSEC], FP8,
                            kind="ExternalOutput")

    with tile.TileContext(nc) as tc:
        with (
            tc.tile_pool(name="w", bufs=1) as wpool,
            tc.tile_pool(name="x", bufs=1) as xpool,
            tc.tile_pool(name="out", bufs=2) as opool,
            tc.tile_pool(name="ps", bufs=2, space="PSUM") as pspool,
        ):
            # banded weights, one tile per unit, on the ACT HWDGE ring so
            # the sync ring starts on x
            wt = []
            for u, (jlo, jhi, kts) in enumerate(units):
                ncols = (jhi - jlo + 1) * OC
                w = wpool.tile([128, len(kts) * ncols], FP8, tag=f"w{u}",
                               name=f"w{u}")
                nc.scalar.dma_start(w[:], wb_d[:, offs[u]:offs[u + 1]])
                wt.append(w)
            nthr = wpool.tile([128, 1], F32, tag="nthr")
            nc.any.memset(nthr[:], -THRESHOLD)
            # x: one tile per (unit, batch quarter), quarter-major issue
            # order so the PE can start as soon as quarter 0 is resident
            xt = [[None] * nU for _ in range(nq)]
            for q in range(nq):
                for u, (_, _, kts) in enumerate(units):
                    if mode == "drsw":
                        t = xpool.tile([128, tpq, 256], FP8,
                                       tag=f"x{q}_{u}", name=f"x{q}_{u}")
                        nc.sync.dma_start(
                            t[:], xT_d[:, u, q * tpq:(q + 1) * tpq, :])
                    else:
                        t = xpool.tile([128, len(kts), qs], FP8,
                                       tag=f"x{q}_{u}", name=f"x{q}_{u}")
                        nc.sync.dma_start(
                            t[:],
                            xT_d[:, kts[0]:kts[-1] + 1, q * qs:(q + 1) * qs])
                    xt[q][u] = t
            po = sp2 = None
            for bt in range(n_bt):
                q, tl = bt // tpq, bt % tpq
                s = bt % ob
                if s == 0:
                    po = opool.tile([128, ob, OC * NSEC], BF16, tag="po")
                    sp2 = opool.tile([128, ob, OC * NSEC], FP8, tag="sp")
                ps = pspool.tile([128, PSUM_COLS], F32, tag="ps")
                for (u, a, b, st, stp) in segs:
                    jlo, jhi, kts = units[u]
                    ncols = (jhi - jlo + 1) * OC
                    pm = None
                    if len(kts) == 2:
                        if mode == "drsw":
                            lhsT = xt[q][u][:, tl, :]
                            pm = mybir.MatmulPerfMode.DoubleRowSwInterleave
                        else:
                            lhsT = xt[q][u][:, :, tl * BT:(tl + 1) * BT]
                            pm = mybir.MatmulPerfMode.DoubleRow
                        wv = wt[u][:].rearrange("p (t n) -> p t n", t=2)
                        rhs = wv[:, :, a - jlo * OC: b - jlo * OC]
                    else:
                        if mode == "drsw":
                            lhsT = xt[q][u][:, tl, 0:128]
                        else:
                            lhsT = xt[q][u][:, 0,
                                            tl * BT:(tl + 1) * BT]
                        rhs = wt[u][:, a - jlo * OC: b - jlo * OC]
                    nc.tensor.matmul(ps[:, a:b], lhsT, rhs,
                                     start=st, stop=stp, perf_mode=pm)
                # [p, i, o, h] view, h innermost -> one reduce_max over X
                psv = ps[:, :NJ * OC].rearrange(
                    "p (i h o) -> p i o h", h=4, o=OC)
                pov = po[:, s, :].rearrange("p (i o) -> p i o", i=NSEC)
                nc.vector.tensor_reduce(
                    pov, psv, axis=mybir.AxisListType.X,
                    op=mybir.AluOpType.max)
                # spks = Relu(Sign(pots - thr)) on ACT
                nc.scalar.activation(
                    sp2[:, s, :], po[:, s, :],
                    mybir.ActivationFunctionType.Sign, bias=nthr[:])
                nc.scalar.activation(
                    sp2[:, s, :], sp2[:, s, :],
                    mybir.ActivationFunctionType.Relu)
                if s == ob - 1:
                    t0 = bt - (ob - 1)
                    nc.scalar.dma_start(
                        pots_d[t0:t0 + ob].rearrange("t p n -> p t n"),
                        po[:])
                    nc.gpsimd.dma_start(
                        spks_d[t0:t0 + ob].rearrange("t p n -> p t n"),
                        sp2[:])
    nc.compile()
    return nc


_PROGRAM_CACHE = {}


def _get_program(bc=BC, mode=None):
    key = (bc, mode or MODE)
    if key not in _PROGRAM_CACHE:
        _PROGRAM_CACHE[key] = _build_program(bc, mode)
    return _PROGRAM_CACHE[key]


def _prep_inputs(x, W, bc=BC, ncores=NCORES, mode=None):
    mode = mode or MODE
    units = _units(mode)
    wb, _, _ = _build_wband(W, units)
    xf = np.asarray(x, np.float32).reshape(-1, E)
    n_bt = bc // BT
    in_maps = []
    for ci in range(ncores):
        xs = xf[ci * bc:(ci + 1) * bc]
        xpad = np.zeros((bc, EP), np.float32)
        xpad[:, :E] = xs
        xq = xpad.astype(NP_FP8)
        if mode == "drsw":
            # [128, nunits, n_bt, 256]: per (unit, batch tile) the stationary
            # pair pre-interleaved + column-reversed as DoubleRowSwInterleave
            # expects: flat f = 2*(127 - m) + t for k-tile t, local sample m.
            xk = xq.reshape(bc, NKT, 128)
            nU = len(units)
            out = np.zeros((128, nU, n_bt, 256), NP_FP8)
            for u, (_, _, kts) in enumerate(units):
                a = xk[:, kts[0], :].T.reshape(128, n_bt, BT)[:, :, ::-1]
                if len(kts) == 2:
                    b = xk[:, kts[1], :].T.reshape(128, n_bt, BT)[:, :, ::-1]
                    st = np.stack([a, b], axis=-1)       # [128, n_bt, 128, 2]
                    out[:, u] = st.reshape(128, n_bt, 256)
                else:
                    out[:, u, :, 0:128] = a[:, :, ::-1]  # plain layout
            in_maps.append({"xT": out, "Wb": wb})
        else:
            # [bc, 13, 128] -> [128, 13, bc]
            xT = np.ascontiguousarray(
                xq.reshape(bc, NKT, 128).transpose(2, 1, 0))
            in_maps.append({"xT": xT, "Wb": wb})
    return in_maps


def kernel(x, W):
    nc = _get_program()
    in_maps = _prep_inputs(x, W)
    res = run_bass_kernel_spmd(nc, in_maps, list(range(NCORES)))
    pots = np.concatenate(
        [np.asarray(r["pots"]).astype(np.float32).reshape(BC, NSEC, OC)
         for r in res.results], axis=0)
    spks = np.concatenate(
        [np.asarray(r["spks"]).astype(np.float32).reshape(BC, NSEC, OC)
         for r in res.results], axis=0)
    pots = np.ascontiguousarray(pots.transpose(0, 2, 1))[..., None]
    spks = np.ascontiguousarray(spks.transpose(0, 2, 1))[..., None]
    return pots, spks
